# revision 8
# baseline (speedup 1.0000x reference)
"""GCMC (NGCF-style) forward on 8 Trainium2 NeuronCores — v2.

Sharding: edges partitioned by destination-row range (18816 rows/core).
Each core computes its row-block of both propagation layers via one-hot
matmul segment-sum (PSUM-accumulated), cores AllGather the updated node
table between layers, and the BPR batch is data-parallel (1024 slots/core)
with a final 2-scalar AllReduce.

v2 over baseline:
  - One-hot scatter matrices M are pure host-known constants -> built on
    the host (bf16) and DMA'd in; the per-chunk DVE is_equal builds are gone.
  - Node tables stored bf16 padded to 256B rows ([NP_,128]); scatter
    matmuls run bf16 (1 cyc/row vs 4 for fp32).
  - dma_gather descriptor generation round-robins SWDGE queues 0..3
    (each queue runs on its own Q7 core pair -> ~4x desc-gen).
  - Leaky ReLU via one Prelu activation (alpha=0.2); fp32 0.8/0.2 hack gone.

Node tables are stored in a host-side permuted ("kpb") layout
  table_row(node r) = (core(r)*128 + (r%18816)%128) * 147 + (r%18816)//128
so each core's computed block is a single contiguous DMA and AllGather
concatenation reproduces the layout. All gather indices are pre-permuted on
the host accordingly.
"""
import numpy as np
import ml_dtypes

import concourse.bass as bass
import concourse.bacc as bacc
import concourse.mybir as mybir
import concourse.tile as tile
from concourse.bass_utils import run_bass_kernel_spmd
from concourse.library_config import mlp as _mlp_lib

U, I, D = 100000, 50000, 64
N = U + I
E = 3_000_000
B = 8192
NEG_SLOPE = 0.2
REG_LAMBDA = 1e-4

NCORES = 8
P = 128
DP = 128                      # padded (256B) table row, elems of bf16
NBLK = 147                    # 128-row blocks per core
RPC = P * NBLK                # 18816 rows per core
NP_ = NCORES * RPC            # 150528 padded node count
NRANGE = 5                    # int16 index ranges of 32768 table rows
RANGE_ROWS = 32768
W_BLOCKS = 3                  # blocks per gather window
MAX_CALL_CHUNKS = 30          # <=3840 idx per dma_gather (>=4096 crashes)
BPC = B // NCORES             # 1024 BPR slots per core
BJ = BPC // P                 # 8 slot groups per core

F32 = mybir.dt.float32
BF16 = mybir.dt.bfloat16
I32 = mybir.dt.int32
I16 = mybir.dt.int16
BF = ml_dtypes.bfloat16
AF = mybir.ActivationFunctionType
ALU = mybir.AluOpType


def _perm(nodes):
    """node id -> row in the kpb-permuted table."""
    k = nodes // RPC
    loc = nodes % RPC
    return (k * P + loc % P) * NBLK + loc // P


def _pack_idx16(flat):
    """flat int16 idx list (len % 128 == 0) -> [128, len/16] dma_gather layout."""
    L = len(flat)
    a = flat.reshape(L // 16, 16).T          # idx i at [i%16, i//16]
    return np.tile(a, (NCORES, 1)).copy()    # replicate to 128 partitions


def prep(inputs):
    """Host-side sharding. Returns (sched, in_maps) where sched drives
    program construction and in_maps has per-core input arrays."""
    rows = np.asarray(inputs["rows"], np.int64)
    cols = np.asarray(inputs["cols"], np.int64)
    vals = np.asarray(inputs["vals"], np.float32)

    cperm = _perm(cols)
    ridx = cperm >> 15                 # range id 0..4
    lidx = (cperm & 32767).astype(np.int16)
    core = rows // RPC

    per_core = []
    cnts = np.zeros((NCORES, NBLK * NRANGE), np.int64)
    for k in range(NCORES):
        m = core == k
        r_loc = rows[m] - k * RPC
        bb = r_loc >> 7
        rr = (r_loc & 127).astype(np.float32)
        key = bb * NRANGE + ridx[m]
        cnts[k] = np.bincount(key, minlength=NBLK * NRANGE)
        per_core.append((key, rr, lidx[m], vals[m]))

    C = -(-cnts.max(axis=0) // P)            # [NBLK*NRANGE] chunks per (b, ri)

    windows = [list(range(s, min(s + W_BLOCKS, NBLK)))
               for s in range(0, NBLK, W_BLOCKS)]
    col_start = np.zeros(NBLK * NRANGE, np.int64)   # first chunk col of (b, ri)
    win_info = []   # per window: (wstart, nw, [(ri, col_off_in_window, nchunks)])
    block_cols = [[] for _ in range(NBLK)]          # per block: chunk cols in order
    pos = 0
    for blocks in windows:
        wstart = pos
        parts = []
        for ri in range(NRANGE):
            n_r = 0
            for b in blocks:
                col_start[b * NRANGE + ri] = pos
                block_cols[b].extend(range(pos, pos + C[b * NRANGE + ri]))
                pos += C[b * NRANGE + ri]
                n_r += C[b * NRANGE + ri]
            parts.append((ri, wstart, n_r))
        off = 0
        parts2 = []
        for ri, _, n_r in parts:
            parts2.append((ri, off, n_r))
            off += n_r
        win_info.append((wstart, pos - wstart, parts2))
    nchunk = pos

    chunk_ri = np.zeros(nchunk, np.int64)
    for bri in range(NBLK * NRANGE):
        cs, cn = col_start[bri], C[bri]
        chunk_ri[cs:cs + cn] = bri % NRANGE
    sched = dict(nchunk=nchunk, windows=windows, win_info=win_info,
                 block_cols=block_cols, chunk_ri=chunk_ri, C=C)

    ego0 = np.concatenate([np.asarray(inputs["user_emb"], np.float32),
                           np.asarray(inputs["item_emb"], np.float32)], axis=0)
    ego0_pad = np.zeros((NP_, D), np.float32)
    ego0_pad[:N] = ego0
    t_of_node = _perm(np.arange(NP_))
    ego0_perm = np.zeros((NP_, D), np.float32)
    ego0_perm[t_of_node] = ego0_pad
    ego0h = np.zeros((NP_, DP), BF)
    ego0h[:, :D] = ego0_perm.astype(BF)

    user = np.asarray(inputs["user"], np.int64)
    pos_i = np.asarray(inputs["positive"], np.int64)
    neg_i = np.asarray(inputs["negative"], np.int64)
    uP = _perm(user).astype(np.int32)
    pP = _perm(U + pos_i).astype(np.int32)
    nP = _perm(U + neg_i).astype(np.int32)

    # fp32 consts: 4 bias columns + fp32 identity (for PE transpose)
    fc = np.concatenate([
        np.asarray(inputs["b_gcn0"], np.float32).T,
        np.asarray(inputs["b_mlp0"], np.float32).T,
        np.asarray(inputs["b_gcn1"], np.float32).T,
        np.asarray(inputs["b_mlp1"], np.float32).T,
        np.eye(D, dtype=np.float32),
    ], axis=1)
    # bf16 consts: weights + identity
    fc16 = np.concatenate([
        np.asarray(inputs["W_gcn0"], np.float32),
        np.asarray(inputs["W_mlp0"], np.float32),
        np.asarray(inputs["W_gcn1"], np.float32),
        np.asarray(inputs["W_mlp1"], np.float32),
        np.eye(D, dtype=np.float32),
    ], axis=1).astype(BF)

    qcols = np.arange(P, dtype=np.float32)[None, None, :]
    in_maps = []
    for k in range(NCORES):
        key, rr, li, vv = per_core[k]
        order = np.argsort(key, kind="stable")
        key_s, rr_s, li_s, vv_s = key[order], rr[order], li[order], vv[order]
        gstart = np.zeros(NBLK * NRANGE + 1, np.int64)
        np.cumsum(np.bincount(key_s, minlength=NBLK * NRANGE), out=gstart[1:])
        within = np.arange(len(key_s)) - gstart[key_s]
        chunkcol = col_start[key_s] + within // P
        lane = within % P

        rows_arr = np.zeros((nchunk, P), np.float32)
        vals_arr = np.zeros((nchunk, P), np.float32)
        idx_arr = np.zeros((nchunk, P), np.int16)
        rows_arr[chunkcol, lane] = rr_s
        vals_arr[chunkcol, lane] = vv_s
        idx_arr[chunkcol, lane] = li_s

        # host-built one-hot scatter matrices: [128 lane, nchunk, 128 q] bf16
        m_all = np.zeros((P, nchunk, P), BF)
        BATCH = 256
        for s in range(0, nchunk, BATCH):
            e = min(nchunk, s + BATCH)
            blk = (rows_arr[s:e, :, None] == qcols) * vals_arr[s:e, :, None]
            m_all[:, s:e, :] = blk.astype(BF).transpose(1, 0, 2)

        idx_all = _pack_idx16(idx_arr.reshape(-1))   # [128, nchunk*8]

        s0 = k * BPC
        bidx = np.concatenate([
            uP[s0:s0 + BPC].reshape(P, BJ),
            pP[s0:s0 + BPC].reshape(P, BJ),
            nP[s0:s0 + BPC].reshape(P, BJ),
        ], axis=1)

        in_maps.append(dict(
            ego0=ego0_perm, ego0h=ego0h,
            m_all=np.ascontiguousarray(m_all.reshape(P, nchunk * P)),
            idx_all=np.ascontiguousarray(idx_all),
            fconst=np.ascontiguousarray(fc),
            fconst16=np.ascontiguousarray(fc16),
            bidx=np.ascontiguousarray(bidx),
        ))
    return sched, in_maps


def build(sched):
    nchunk = sched["nchunk"]
    win_info = sched["win_info"]
    windows = sched["windows"]
    block_cols = sched["block_cols"]
    max_nw = max(nw for _, nw, _ in win_info)

    nc = bacc.Bacc(num_swdge_queues=4)
    ego0 = nc.dram_tensor("ego0", [NP_, D], F32, kind="ExternalInput")
    ego0h = nc.dram_tensor("ego0h", [NP_, DP], BF16, kind="ExternalInput")
    m_all = nc.dram_tensor("m_all", [P, nchunk * P], BF16, kind="ExternalInput")
    idx_all = nc.dram_tensor("idx_all", [P, nchunk * 8], I16, kind="ExternalInput")
    fconst = nc.dram_tensor("fconst", [D, 4 + D], F32, kind="ExternalInput")
    fconst16 = nc.dram_tensor("fconst16", [D, 4 * D + D], BF16,
                              kind="ExternalInput")
    bidx = nc.dram_tensor("bidx", [P, 3 * BJ], I32, kind="ExternalInput")
    out_ext = nc.dram_tensor("out", [1, 2], F32, kind="ExternalOutput")

    ego_blk = [nc.dram_tensor(f"ego{l}_blk", [RPC, DP], BF16) for l in (1, 2)]
    ego_full = [nc.dram_tensor(f"ego{l}_full", [NP_, DP], BF16,
                               addr_space="Shared") for l in (1, 2)]
    ego_loc = [nc.dram_tensor(f"ego{l}_loc", [NP_, DP], BF16) for l in (1, 2)]
    ar_in = nc.dram_tensor("ar_in", [1, 8], F32)
    ar_out = nc.dram_tensor("ar_out", [1, 8], F32, addr_space="Shared")

    RGRP = [list(range(NCORES))]
    qctr = [0]

    def next_q():
        q = qctr[0] % 4
        qctr[0] += 1
        return q

    with tile.TileContext(nc) as tc:
        nc.gpsimd.load_library(_mlp_lib)
        with (
            tc.tile_pool(name="const", bufs=1) as cp,
            tc.tile_pool(name="sb", bufs=3) as sp,
            tc.tile_pool(name="gp", bufs=4) as gp,
            tc.tile_pool(name="mp", bufs=2) as mp,
            tc.tile_pool(name="pp", bufs=2, space="PSUM") as pp,
        ):
            fc_sb = cp.tile([D, 4 + D], F32)
            nc.sync.dma_start(fc_sb[:], fconst[:])
            fc16_sb = cp.tile([D, 4 * D + D], BF16)
            nc.sync.dma_start(fc16_sb[:], fconst16[:])
            bidx_sb = cp.tile([P, 3 * BJ], I32)
            nc.sync.dma_start(bidx_sb[:], bidx[:])

            w_g = [fc16_sb[:, 0:D], fc16_sb[:, 2 * D:3 * D]]
            w_m = [fc16_sb[:, D:2 * D], fc16_sb[:, 3 * D:4 * D]]
            ident = fc_sb[:, 4:4 + D]
            ident16 = fc16_sb[:, 4 * D:5 * D]
            bg = [fc_sb[:, 0:1], fc_sb[:, 2:3]]
            bm = [fc_sb[:, 1:2], fc_sb[:, 3:4]]

            ego_nat = cp.tile([P, NBLK, DP], BF16)
            nc.vector.memset(ego_nat[:], 0.0)

            scratch = pp.tile([1, 1], F32, tag="scr", bufs=1)
            nc.tensor.matmul(scratch[:], lhsT=fc16_sb[:, 0:1],
                             rhs=fc16_sb[:, 0:1], start=True, stop=True)

            # ---- BPR gathers + per-layer stats ----------------------------
            gb = {}
            ss = {}
            dp = {}
            dn = {}

            def bpr_layer(l, table, dt, w):
                """Gather u/p/n rows of `table` for this core's 1024 slots and
                compute per-slot norms (Square accum) and u.p / u.n dots.
                Rows are gathered at full width w; only [0:D] is data."""
                for role in range(3):
                    g = cp.tile([P, BJ, w], dt, name=f"gb{l}_{role}")
                    for j in range(BJ):
                        nc.gpsimd.indirect_dma_start(
                            out=g[:, j, :], out_offset=None, in_=table,
                            in_offset=bass.IndirectOffsetOnAxis(
                                ap=bidx_sb[:, role * BJ + j:role * BJ + j + 1],
                                axis=0))
                    gb[(l, role)] = g
                for role in range(3):
                    s = cp.tile([P, BJ], F32, name=f"ss{l}_{role}")
                    for j in range(BJ):
                        sq = sp.tile([P, D], F32, tag="sqscr")
                        nc.scalar.activation(sq[:], gb[(l, role)][:, j, 0:D],
                                             AF.Square, accum_out=s[:, j:j + 1])
                    ss[(l, role)] = s
                for role, dst in ((1, dp), (2, dn)):
                    d = cp.tile([P, BJ], F32, name=f"d{l}_{role}")
                    for j in range(BJ):
                        m = sp.tile([P, D], F32, tag="dotscr")
                        nc.vector.tensor_tensor(m[:], gb[(l, 0)][:, j, 0:D],
                                                gb[(l, role)][:, j, 0:D],
                                                ALU.mult)
                        nc.vector.tensor_reduce(d[:, j:j + 1], m[:],
                                                mybir.AxisListType.X, ALU.add)
                    dst[l] = d

            bpr_layer(0, ego0[:], F32, D)

            # ---- propagation layers --------------------------------------
            for l in range(2):
                table = ego0h if l == 0 else ego_loc[0]
                for wi, blocks in enumerate(windows):
                    wstart, nw, parts = win_info[wi]
                    idx_w = sp.tile([P, max_nw * 8], I16, tag="idxw")
                    nc.sync.dma_start(idx_w[:, :nw * 8],
                                      idx_all[:, wstart * 8:(wstart + nw) * 8])
                    m_w = mp.tile([P, max_nw, P], BF16, tag="M")
                    nc.sync.dma_start(
                        m_w[:, :nw, :].rearrange("p a b -> p (a b)"),
                        m_all[:, wstart * P:(wstart + nw) * P])
                    G = gp.tile([P, max_nw, DP], BF16, tag="G")
                    for ri, off, n_r in parts:
                        lo = ri * RANGE_ROWS
                        hi = min(NP_, lo + RANGE_ROWS)
                        # queue balance: ranges 0-3 pin to queues 0-3; the
                        # 5th range is split across all four queues.
                        if ri < 4 and n_r <= MAX_CALL_CHUNKS:
                            pieces = [(off, n_r, ri)]
                        else:
                            k4 = -(-n_r // 4)
                            pieces = [(off + j * k4, min(k4, n_r - j * k4), j)
                                      for j in range(4) if n_r - j * k4 > 0]
                        for o, n_s, q in pieces:
                            nc.gpsimd.dma_gather(
                                out_ap=G[:, o:o + n_s, :],
                                in_ap=table[lo:hi, :],
                                idxs_ap=idx_w[:, o * 8:(o + n_s) * 8],
                                num_idxs=n_s * P, num_idxs_reg=n_s * P,
                                elem_size=DP, single_packet=False,
                                queue_num=q,
                            )
                    # PE touch: absorb the gather/M waits once per window
                    nc.tensor.matmul(scratch[:], lhsT=G[:, 0, 0:1],
                                     rhs=m_w[:, 0, 0:1], start=True, stop=True)
                    for b in blocks:
                        cols_b = block_cols[b]
                        psum_side = pp.tile([D, P], F32, tag="side")
                        nchunks_b = len(cols_b)
                        for ci, col in enumerate(cols_b):
                            nc.tensor.matmul(
                                psum_side[:], lhsT=G[:, col - wstart, 0:D],
                                rhs=m_w[:, col - wstart, :],
                                start=(ci == 0), stop=(ci == nchunks_b - 1))
                        sideT = sp.tile([D, P], BF16, tag="sideT")
                        nc.scalar.copy(sideT[:], psum_side[:])
                        p1 = pp.tile([D, P], F32, tag="dense")
                        nc.tensor.matmul(p1[:], lhsT=w_g[l], rhs=sideT[:],
                                         start=True, stop=True)
                        gcnT = sp.tile([D, P], BF16, tag="gcnT")
                        nc.scalar.activation(gcnT[:], p1[:], AF.Prelu,
                                             bias=bg[l], alpha=NEG_SLOPE)
                        p2 = pp.tile([D, P], F32, tag="dense")
                        nc.tensor.matmul(p2[:], lhsT=w_m[l], rhs=gcnT[:],
                                         start=True, stop=True)
                        egoT = sp.tile([D, P], BF16, tag="egoT")
                        nc.scalar.activation(egoT[:], p2[:], AF.Identity,
                                             bias=bm[l])
                        p3 = pp.tile([P, D], BF16, tag="p3")
                        nc.tensor.transpose(p3[:], egoT[:], ident16)
                        nc.scalar.copy(ego_nat[:, b, 0:D], p3[:])

                nc.sync.dma_start(
                    ego_blk[l][:].rearrange("(p r) d -> p (r d)", p=P),
                    ego_nat[:].rearrange("p r d -> p (r d)"))
                nc.gpsimd.collective_compute(
                    "AllGather", ALU.bypass, replica_groups=RGRP,
                    ins=[ego_blk[l][:]], outs=[ego_full[l][:]])
                nc.sync.dma_start(
                    ego_loc[l][:].rearrange("(a b) d -> a (b d)", a=P),
                    ego_full[l][:].rearrange("(a b) d -> a (b d)", a=P))
                bpr_layer(l + 1, ego_loc[l][:], BF16, DP)

            # ---- final combine -------------------------------------------
            def norm_term(d, su, so):
                t = sp.tile([P, BJ], F32, tag="nt", bufs=6)
                nc.vector.tensor_tensor(t[:], su[:], so[:], ALU.mult)
                t2 = sp.tile([P, BJ], F32, tag="nt", bufs=6)
                nc.scalar.activation(t2[:], t[:], AF.Sqrt)
                t3 = sp.tile([P, BJ], F32, tag="nt", bufs=6)
                nc.vector.reciprocal(t3[:], t2[:])
                t4 = sp.tile([P, BJ], F32, tag="nt", bufs=6)
                nc.vector.tensor_tensor(t4[:], d[:], t3[:], ALU.mult)
                return t4

            pos_s = cp.tile([P, BJ], F32)
            nc.vector.tensor_tensor(pos_s[:], dp[0][:],
                                    norm_term(dp[1], ss[(1, 0)], ss[(1, 1)])[:],
                                    ALU.add)
            nc.vector.tensor_tensor(pos_s[:], pos_s[:],
                                    norm_term(dp[2], ss[(2, 0)], ss[(2, 1)])[:],
                                    ALU.add)
            neg_s = cp.tile([P, BJ], F32)
            nc.vector.tensor_tensor(neg_s[:], dn[0][:],
                                    norm_term(dn[1], ss[(1, 0)], ss[(1, 2)])[:],
                                    ALU.add)
            nc.vector.tensor_tensor(neg_s[:], neg_s[:],
                                    norm_term(dn[2], ss[(2, 0)], ss[(2, 2)])[:],
                                    ALU.add)
            xdiff = cp.tile([P, BJ], F32)
            nc.vector.tensor_tensor(xdiff[:], neg_s[:], pos_s[:], ALU.subtract)
            ex = cp.tile([P, BJ], F32)
            nc.scalar.activation(ex[:], xdiff[:], AF.Exp)
            sp_ = cp.tile([P, BJ], F32)
            nc.scalar.activation(sp_[:], ex[:], AF.Ln, bias=1.0)

            reg_row = cp.tile([P, BJ], F32)
            nc.vector.tensor_tensor(reg_row[:], ss[(0, 0)][:], ss[(0, 1)][:],
                                    ALU.add)
            nc.vector.tensor_tensor(reg_row[:], reg_row[:], ss[(0, 2)][:],
                                    ALU.add)

            sc = cp.tile([P, 2], F32)
            srow = cp.tile([P, 1], F32)
            nc.vector.tensor_reduce(srow[:], sp_[:], mybir.AxisListType.X, ALU.add)
            nc.scalar.activation(sc[:, 0:1], srow[:], AF.Copy, scale=1.0 / B)
            rrow = cp.tile([P, 1], F32)
            nc.vector.tensor_reduce(rrow[:], reg_row[:], mybir.AxisListType.X,
                                    ALU.add)
            nc.scalar.activation(sc[:, 1:2], rrow[:], AF.Copy,
                                 scale=REG_LAMBDA * 0.5 / B)
            ones = cp.tile([P, 1], F32)
            nc.vector.memset(ones[:], 1.0)
            tot = pp.tile([1, 2], F32, tag="tot", bufs=1)
            nc.tensor.matmul(tot[:], lhsT=ones[:], rhs=sc[:], start=True,
                             stop=True)
            ar_sb = cp.tile([1, 8], F32)
            nc.vector.memset(ar_sb[:], 0.0)
            nc.scalar.copy(ar_sb[:, 0:2], tot[:])
            nc.sync.dma_start(ar_in[:], ar_sb[:])
            nc.gpsimd.collective_compute(
                "AllReduce", ALU.add, replica_groups=RGRP,
                ins=[ar_in[:]], outs=[ar_out[:]])
            nc.sync.dma_start(out_ext[:], ar_out[:1, 0:2])
    nc.compile()
    return nc


def run(inputs, trace=False, trace_cores=None):
    inputs = {k: np.asarray(v) for k, v in inputs.items()}
    sched, in_maps = prep(inputs)
    nc = build(sched)
    kw = {}
    if trace:
        kw = dict(trace=True, trace_cores=trace_cores or list(range(NCORES)))
    res = run_bass_kernel_spmd(nc, in_maps, list(range(NCORES)), **kw)
    out = res.results[0]["out"].reshape(2).astype(np.float32)
    return out, res


def kernel(**inputs):
    out, _ = run(inputs)
    return out
